# revision 63
# baseline (speedup 1.0000x reference)
"""Deformable conv2d (3x3, pad 1) on 8 trn2 NeuronCores.

Sharding: (batch b, image half) -> core 2*b + half. Each core:
  1. offset conv (PE matmuls over 2 c-tiles x 9 taps, N=400 position chunks),
     software-pipelined: rc0/rc1 -> block-0 index math -> rc2..rc7 -> rest,
     so block-0 gathers start ~35us in, under the conv tail
  2. transpose offsets to position-major; DVE index/bilinear-weight math;
     idx fold to the gather's 16-wrapped layout via 8 PE permute-matmuls
     (fp32 exact) + strided DVE copies -- all on-chip, no DMA bounce
  3. dma_gather of 2x2 fp16 pixel patches (all 256 ch) from an interleaved
     row-pair HBM image, positions-on-partitions; gather buffers live in a
     pool that coexists with the prologue pool (no SBUF-reuse barrier)
  4. bilinear combine on PE as diagonal-matmul accumulation
     (psum[c,p] += sum_j plane_j^T @ diag(beta_j)) -- transposes to
     channel-major for free; the 36 diag tiles per chunk are built in ONE
     stride-0-broadcast DVE op (dt_all = id16 * beta16)
  5. main conv = 18-chunk PE accumulation over (c-tile, tap); bias folded
     into the per-partition activation bias of the psum->SBUF copy; fp16
     output staging (host upcasts).
"""
import numpy as np

B, CIN, COUT, H, W = 4, 256, 256, 80, 80
NCORES = 8
HHALF = H // 2                      # 40 rows per core
NPOS = HHALF * W                    # 3200 positions per core
NCHUNK = NPOS // 128                # 25
PITCH = 84                          # x2 pixel-group pitch per row
NGROUPS = PITCH * PITCH             # 7056 (rows in x2; 83*84 + slack)
FBIAS = 16.0                        # float->int truncation bias (floor trick)
CLIP_LO = 14.0                      # = -2 + FBIAS
CLIP_HI = 96.9                      # = 80.9 + FBIAS
FLAT_OFF = -(14 * PITCH + 14)       # flat = y0b*84 + x0b + FLAT_OFF
BLOCKS = [(0, 384), (384, 512), (896, 512), (1408, 512),
          (1920, 512), (2432, 512), (2944, 256)]
NT_PE = 9                           # taps 0..NT_PE-1 combine on PE (diag)

_cached = {}


def _build_program():
    from concourse import bass, bacc, tile, mybir
    from contextlib import ExitStack

    fp16, fp32 = mybir.dt.float16, mybir.dt.float32
    i16, i32 = mybir.dt.int16, mybir.dt.int32
    A = mybir.AluOpType
    ACT_COPY = mybir.ActivationFunctionType.Copy
    ACT_IDENT = mybir.ActivationFunctionType.Identity

    nc = bacc.Bacc("TRN2", target_bir_lowering=False, debug=False,
                   num_devices=NCORES, num_swdge_queues=4)

    x2_d = nc.dram_tensor("x2", [NGROUPS, 512], fp16, kind="ExternalInput")
    xcf_d = nc.dram_tensor("xcf", [128, 2, 44 * PITCH], fp16, kind="ExternalInput")
    offw_d = nc.dram_tensor("offw", [128, 2, 9, 18], fp16, kind="ExternalInput")
    offb_d = nc.dram_tensor("offb", [18, 1], fp32, kind="ExternalInput")
    convw_d = nc.dram_tensor("convw", [128, 2, 9, 256], fp16, kind="ExternalInput")
    convb_d = nc.dram_tensor("convb", [128, 2], fp32, kind="ExternalInput")
    cyb_d = nc.dram_tensor("cyb", [128, NCHUNK, 9], fp32, kind="ExternalInput")
    cxb_d = nc.dram_tensor("cxb", [128, NCHUNK, 9], fp32, kind="ExternalInput")
    id16_d = nc.dram_tensor("id16", [128, 128], fp16, kind="ExternalInput")
    id32_d = nc.dram_tensor("id32", [18, 18], fp32, kind="ExternalInput")
    perm_d = nc.dram_tensor("perm", [128, 8, 128], fp32, kind="ExternalInput")
    out_d = nc.dram_tensor("out", [2, 128, NPOS], fp16, kind="ExternalOutput")

    # overlapping gather-source view: [NGROUPS-1, 1024] with row stride 512
    x2_view = x2_d.ap().copy()
    v = x2_view.ap
    v[0] = [512, NGROUPS - 1]
    v[1] = [1, 1024]
    x2_view.ap = v

    def revec(ap, dims, extra_offset=0):
        """Rebuild an AP's dim list: dims = [(stride, num), ...]."""
        a = ap.copy()
        vv = a.ap
        while len(vv) > 1:
            vv.pop()
        vv[0] = list(dims[0])
        for d in dims[1:]:
            vv.append(list(d))
        a.ap = vv
        a.offset = a.offset + extra_offset
        return a

    with tile.TileContext(nc) as tc:
        with ExitStack() as ctx:
            persist = ctx.enter_context(tc.tile_pool(name="persist", bufs=1))
            idxw = persist.tile([128, 9, NPOS // 16], i16)
            beta16 = persist.tile([128, NCHUNK, 9, 4], fp16)
            id16 = persist.tile([128, 128], fp16)
            convw = persist.tile([128, 2, 9, 256], fp16)
            convb = persist.tile([128, 2], fp32)
            # gather buffers outlive the prologue pool so block-0 gathers
            # can run under the tail of the offset conv (no SBUF-reuse
            # barrier against prologue tiles)
            gbpool = ctx.enter_context(tc.tile_pool(name="gbp", bufs=1))

            # ---------------- prologue: offsets + indices ----------------
            with ExitStack() as pctx:
                ppool = pctx.enter_context(tc.tile_pool(name="pro", bufs=1))
                ppsum = pctx.enter_context(
                    tc.tile_pool(name="ppsum", bufs=2, space="PSUM"))

                xcf_a = ppool.tile([128, 2, 14 * PITCH], fp16)
                xcf_b = ppool.tile([128, 2, 33 * PITCH], fp16)
                offw = ppool.tile([128, 2, 9, 18], fp16)
                offb = ppool.tile([18, 1], fp32)
                id32 = ppool.tile([18, 18], fp32)
                cyb = ppool.tile([128, NCHUNK, 9], fp32)
                cxb = ppool.tile([128, NCHUNK, 9], fp32)
                perm = ppool.tile([128, 8, 128], fp32)
                off_sb = ppool.tile([18, NPOS], fp32)
                offT = ppool.tile([128, NCHUNK, 18], fp32)
                # conv-critical loads first; everything else after
                nc.sync.dma_start(out=offw[:], in_=offw_d[:])
                nc.sync.dma_start(out=offb[:], in_=offb_d[:])
                nc.sync.dma_start(out=xcf_a[:], in_=xcf_d[:, :, 0:14 * PITCH])
                nc.sync.dma_start(out=xcf_b[:],
                                  in_=xcf_d[:, :, 11 * PITCH:44 * PITCH])
                nc.sync.dma_start(out=id32[:], in_=id32_d[:])
                nc.sync.dma_start(out=cyb[:], in_=cyb_d[:])
                nc.sync.dma_start(out=cxb[:], in_=cxb_d[:])
                nc.sync.dma_start(out=perm[:], in_=perm_d[:])
                nc.sync.dma_start(out=id16[:], in_=id16_d[:])
                nc.sync.dma_start(out=convw[:], in_=convw_d[:])
                nc.sync.dma_start(out=convb[:], in_=convb_d[:])

                # offset conv: 8 chunks x 5 rows x 80 cols (N=400)
                # core's rows start at h0 (baked into cyb); xcf rows are
                # sample-global, so the row window ALSO must come from host:
                # we bake it by sending xcf pre-SLICED? No: xcf is full padded
                # image; row base differs per core. Use a per-core scalar via
                # cyb? Instead: host sends same xcf; the row offset is encoded
                # in a dram-input scalar-free way: we read rows via h0 from
                # cyb is not an AP offset. Trick: host sends xcf with the
                # core's 44-row window ALREADY positioned at a fixed place:
                # xcf layout = padded rows [h0-2 .. h0+42) relocated to rows
                # [0..44). See host prep. Conv output row r (0..39) reads
                # xcf rows (r + ky) in [0..42).
                NE = NCHUNK * 9   # 225
                pyb = ppool.tile([128, NE], fp32)
                pxb = ppool.tile([128, NE], fp32)
                t_i32 = ppool.tile([128, NE], i32)
                y0f = ppool.tile([128, NE], fp32)
                x0f = ppool.tile([128, NE], fp32)
                fy = ppool.tile([128, NE], fp32)
                fx = ppool.tile([128, NE], fp32)
                gy = ppool.tile([128, NE], fp32)
                gx = ppool.tile([128, NE], fp32)
                gtt = ppool.tile([128, NE], fp32)

                def conv_rc(rc):
                    ps = ppsum.tile([18, 400], fp32, tag="offps")
                    xt, roff, rows = ((xcf_a, 0, 14) if rc < 2 else
                                      (xcf_b, 11 * PITCH, 33))
                    mm = 0
                    for ct in range(2):
                        for t in range(9):
                            ky, kx = t // 3, t % 3
                            cb = (rc * 5 + ky + 1) * PITCH + kx + 1 - roff
                            mov = revec(xt[:, ct, 0],
                                        [(2 * rows * PITCH, 128), (PITCH, 5),
                                         (1, 80)],
                                        extra_offset=cb)
                            nc.tensor.matmul(
                                ps[:], lhsT=offw[:, ct, t, :], rhs=mov,
                                start=(mm == 0), stop=(mm == 17))
                            mm += 1
                    nc.scalar.activation(off_sb[:, rc * 400:(rc + 1) * 400],
                                         ps[:], ACT_IDENT, bias=offb[:])

                def transp(c0, c1):
                    # offsets [18, 128] -> [128, 18] per chunk
                    for ch in range(c0, c1):
                        ptx = ppsum.tile([128, 18], fp32, tag="offtps", bufs=1)
                        nc.tensor.matmul(ptx[:],
                                         lhsT=off_sb[:, ch * 128:(ch + 1) * 128],
                                         rhs=id32[:], start=True, stop=True,
                                         is_transpose=True)
                        nc.vector.tensor_copy(offT[:, ch, :], ptx[:])

                def math_fold(c0, c1):
                    """index/bilinear-weight math + idx fold, chunks [c0,c1)"""
                    nch = c1 - c0
                    ne = nch * 9
                    fw = lambda t: revec(t[:], [(NE, 128), (1, ne)], c0 * 9)
                    V = nc.vector
                    dyw = revec(offT[:], [(NCHUNK * 18, 128), (18, nch), (2, 9)],
                                c0 * 18)
                    dxw = revec(offT[:], [(NCHUNK * 18, 128), (18, nch), (2, 9)],
                                c0 * 18 + 1)
                    V.tensor_tensor(out=fw(pyb), in0=dyw, in1=cyb[:, c0:c1, :],
                                    op=A.add)
                    V.tensor_tensor(out=fw(pxb), in0=dxw, in1=cxb[:, c0:c1, :],
                                    op=A.add)
                    V.tensor_scalar(fw(pyb), fw(pyb), CLIP_LO, CLIP_HI,
                                    A.max, A.min)
                    V.tensor_scalar(fw(pxb), fw(pxb), CLIP_LO, CLIP_HI,
                                    A.max, A.min)
                    # robust floor: y0 = cvt(pyb); y0 -= (y0 > pyb)
                    V.tensor_copy(fw(t_i32), fw(pyb))
                    V.tensor_copy(fw(y0f), fw(t_i32))
                    V.tensor_tensor(out=fw(gtt), in0=fw(y0f), in1=fw(pyb),
                                    op=A.is_gt)
                    V.tensor_tensor(out=fw(y0f), in0=fw(y0f), in1=fw(gtt),
                                    op=A.subtract)
                    V.tensor_copy(fw(t_i32), fw(pxb))
                    V.tensor_copy(fw(x0f), fw(t_i32))
                    V.tensor_tensor(out=fw(gtt), in0=fw(x0f), in1=fw(pxb),
                                    op=A.is_gt)
                    V.tensor_tensor(out=fw(x0f), in0=fw(x0f), in1=fw(gtt),
                                    op=A.subtract)
                    V.tensor_tensor(out=fw(fy), in0=fw(pyb), in1=fw(y0f),
                                    op=A.subtract)
                    V.tensor_tensor(out=fw(fx), in0=fw(pxb), in1=fw(x0f),
                                    op=A.subtract)
                    # flat = (y0b*84 + x0b) - 1190
                    V.scalar_tensor_tensor(fw(pyb), fw(y0f), float(PITCH),
                                           fw(x0f), A.mult, A.add)
                    V.tensor_scalar_add(fw(pyb), fw(pyb), float(FLAT_OFF))
                    V.tensor_scalar(fw(gy), fw(fy), -1.0, 1.0, A.mult, A.add)
                    V.tensor_scalar(fw(gx), fw(fx), -1.0, 1.0, A.mult, A.add)
                    # beta: b0=gx*gy b1=gx*fy b2=fx*gy b3=fx*fy
                    v3w = lambda t: revec(t[:], [(NE, 128), (9, nch), (1, 9)],
                                          c0 * 9)
                    bjw = lambda j: revec(beta16[:], [(NE * 4, 128), (36, nch),
                                                      (4, 9)], c0 * 36 + j)
                    V.tensor_tensor(out=bjw(0), in0=v3w(gx), in1=v3w(gy), op=A.mult)
                    V.tensor_tensor(out=bjw(1), in0=v3w(gx), in1=v3w(fy), op=A.mult)
                    V.tensor_tensor(out=bjw(2), in0=v3w(fx), in1=v3w(gy), op=A.mult)
                    V.tensor_tensor(out=bjw(3), in0=v3w(fx), in1=v3w(fy), op=A.mult)
                    # idx fold on-chip: flat fp32 -> idxw i16,
                    # idxw[16q+r, k, ch*8+a] = flat[16a+r, ch, k] via PE
                    # permute-matmul (fp32 exact), psum->i32, strided i32->i16.
                    for a in range(8):
                        fpp = ppsum.tile([128, NE], fp32, tag="foldps", bufs=1)
                        nc.tensor.matmul(fpp[:, :ne], lhsT=perm[:, a, :],
                                         rhs=fw(pyb), start=True, stop=True)
                        f32 = ppool.tile([128, NE], i32, tag="ftmp", bufs=2)
                        nc.vector.tensor_copy(f32[:, :ne], fpp[:, :ne])
                        src = revec(f32[:], [(NE, 128), (9, nch), (1, 9)])
                        dst = revec(idxw[:], [(9 * 200, 128), (8, nch), (200, 9)],
                                    a + c0 * 8)
                        nc.vector.tensor_copy(dst, src)

                # pipelined prologue: block 0's indices first so its gathers
                # start while the rest of the offset conv still runs on PE
                conv_rc(0)
                transp(0, 3)
                math_fold(0, 3)
                for rc in range(1, 8):
                    conv_rc(rc)
                transp(3, NCHUNK)
                math_fold(3, NCHUNK)

            # ---------------- main loop ----------------
            with ExitStack() as mctx:
                mpool = mctx.enter_context(tc.tile_pool(name="main", bufs=1))
                mpsum = mctx.enter_context(
                    tc.tile_pool(name="mpsum", bufs=2, space="PSUM"))

                for bi, (base, npos) in enumerate(BLOCKS):
                    nsub = npos // 128
                    gbs = []
                    for k in range(9):
                        gb = gbpool.tile([128, 4, 1024], fp16, tag=f"gb{k}",
                                         bufs=(2 if k < 8 else 1))
                        nc.gpsimd.dma_gather(
                            gb[:, :nsub, :], x2_view,
                            idxw[:, k, base // 16:(base + npos) // 16],
                            npos, npos, 1024, elem_step=512,
                            queue_num=k % 4)
                        gbs.append(gb)

                    # valbuf s-major: [128, s, ci=k*2+ct, 128] so each
                    # (k, s) combine write is one contiguous [128, 256]
                    valbuf = mpool.tile([128, 4, 18, 128], fp16, tag="valbuf",
                                        bufs=2)
                    for s in range(nsub):
                        ch = base // 128 + s
                        # diag tiles, one stride-0 broadcast DVE op:
                        # dt_all[p, k*4+j, :] = id16[p, :] * beta16[p, ch, k, j]
                        dt_all = mpool.tile([128, 36, 128], fp16,
                                            tag="dtall", bufs=2)
                        nc.vector.tensor_tensor(
                            out=dt_all[:],
                            in0=revec(id16[:], [(128, 128), (0, 36), (1, 128)]),
                            in1=revec(beta16[:], [(NE * 4, 128), (1, 36),
                                                  (0, 128)], ch * 36),
                            op=A.mult)
                        # psum[c, p'] += sum_j gb_j^T @ diag(beta_j)
                        # (transpose + scale in one)
                        for k in range(9):
                            pv = mpsum.tile([128, 256], fp32, tag="pvb",
                                            bufs=4)
                            for ct in range(2):
                                for j in range(4):
                                    nc.tensor.matmul(
                                        pv[:, ct * 128:ct * 128 + 128],
                                        lhsT=gbs[k][:, s,
                                                    (2 * j + ct) * 128:
                                                    (2 * j + ct + 1) * 128],
                                        rhs=dt_all[:, k * 4 + j, :],
                                        start=(j == 0), stop=(j == 3))
                            nc.scalar.activation(
                                valbuf[:, s, k * 2:k * 2 + 2, :],
                                pv[:], ACT_COPY)

                    for ot in range(2):
                        po = mpsum.tile([128, 512], fp32, tag=f"po{ot}", bufs=2)
                        for ci in range(18):
                            k, ct = ci // 2, ci % 2
                            rhsv = revec(
                                valbuf[:], [(4 * 18 * 128, 128),
                                            (18 * 128, nsub), (1, 128)],
                                ci * 128)
                            nc.tensor.matmul(
                                po[:, :npos],
                                lhsT=convw[:, ct, k, ot * 128:(ot + 1) * 128],
                                rhs=rhsv,
                                start=(ci == 0), stop=(ci == 17))
                        osb = mpool.tile([128, 512], fp16, tag="osb", bufs=2)
                        nc.scalar.activation(osb[:, :npos], po[:, :npos],
                                             ACT_IDENT, bias=convb[:, ot:ot + 1])
                        nc.sync.dma_start(
                            out=out_d[ot, :, base:base + npos],
                            in_=osb[:, :npos])

    nc.compile()
    return nc


def _host_prep(x, offset_w, offset_b, conv_w, conv_b):
    """Build per-core input maps."""
    x = np.asarray(x, np.float32)
    offset_w = np.asarray(offset_w, np.float32)
    offset_b = np.asarray(offset_b, np.float32)
    conv_w = np.asarray(conv_w, np.float32)
    conv_b = np.asarray(conv_b, np.float32)

    # weights, shared
    # offset_w: [18, 256, 3, 3] -> [c128, ct, t, d]
    ow = offset_w.reshape(18, 2, 128, 3, 3)
    offw_h = np.ascontiguousarray(
        ow.reshape(18, 2, 128, 9).transpose(2, 1, 3, 0)).astype(np.float16)
    offb_h = offset_b.reshape(18, 1).astype(np.float32)
    cw = conv_w.reshape(256, 2, 128, 9)
    convw_h = np.ascontiguousarray(cw.transpose(2, 1, 3, 0)).astype(np.float16)  # [c,ct,t,o]
    convb_h = np.ascontiguousarray(conv_b.reshape(2, 128).T).astype(np.float32)
    id16_h = np.eye(128, dtype=np.float16)
    id32_h = np.eye(18, dtype=np.float32)
    # fold permutation: perm[a][16a+r, 16q+r] = 1  (lhsT for out_a = P_a @ flat)
    perm_h = np.zeros((128, 8, 128), np.float32)
    for a in range(8):
        for r in range(16):
            for q in range(8):
                perm_h[16 * a + r, a, 16 * q + r] = 1.0

    # per-core base constants
    k = np.arange(9)
    ry = (k // 3 - 1).astype(np.float32)
    rx = (k % 3 - 1).astype(np.float32)
    in_maps = []
    per_sample = {}
    for b in range(B):
        xc = np.ascontiguousarray(x[b].transpose(1, 2, 0))       # [H, W, C]
        xp = np.pad(xc, ((2, 2), (2, 2), (0, 0))).astype(np.float16)  # [84, 84, 256]
        x2 = np.zeros((PITCH, PITCH, 2, 256), np.float16)
        x2[:83, :, 0] = xp[:83]
        x2[:83, :, 1] = xp[1:84]
        x2_h = x2.reshape(NGROUPS, 512)
        per_sample[b] = (x2_h, xp)

    for core in range(NCORES):
        b, half = core // 2, core % 2
        h0 = half * HHALF
        x2_h, xp = per_sample[b]
        # xcf: channel-first, rows [h0-2 .. h0+42) of the padded image
        # relocated to local rows [0..44): xcf[c, r, x'] = xpad_cf[c, h0+r, x']
        # padded row index of original row y is y+2; window r=0 -> orig h0-2.
        xcf_rows = xp[h0:h0 + 44]                                # [44, 84, 256]
        xcf_h = np.ascontiguousarray(
            xcf_rows.transpose(2, 0, 1).reshape(2, 128, 44 * PITCH)
            .transpose(1, 0, 2))

        i = np.arange(NPOS)
        hloc = i // W
        wloc = i % W
        cyb_h = ((h0 + hloc)[:, None] + ry[None, :] + FBIAS).astype(np.float32)
        cxb_h = (wloc[:, None] + rx[None, :] + FBIAS).astype(np.float32)
        cyb_h = np.ascontiguousarray(
            cyb_h.reshape(NCHUNK, 128, 9).transpose(1, 0, 2))
        cxb_h = np.ascontiguousarray(
            cxb_h.reshape(NCHUNK, 128, 9).transpose(1, 0, 2))

        in_maps.append({
            "x2": x2_h, "xcf": xcf_h, "offw": offw_h, "offb": offb_h,
            "convw": convw_h, "convb": convb_h, "cyb": cyb_h, "cxb": cxb_h,
            "id16": id16_h, "id32": id32_h, "perm": perm_h,
        })
    return in_maps


def kernel(x, offset_w, offset_b, conv_w, conv_b, _trace=False):
    from concourse.bass_utils import run_bass_kernel_spmd

    if "nc" not in _cached:
        _cached["nc"] = _build_program()
    nc = _cached["nc"]
    in_maps = _host_prep(x, offset_w, offset_b, conv_w, conv_b)
    res = run_bass_kernel_spmd(nc, in_maps, list(range(NCORES)), trace=_trace)
    _cached["last_result"] = res
    out = np.zeros((B, COUT, H, W), np.float32)
    for core in range(NCORES):
        b, half = core // 2, core % 2
        o = res.results[core]["out"]          # [2, 128, NPOS]
        out[b, :, half * HHALF:(half + 1) * HHALF, :] = \
            o.reshape(COUT, HHALF, W)
    return out



# revision 64
# speedup vs baseline: 1.0845x; 1.0845x over previous
"""Deformable conv2d (3x3, pad 1) on 8 trn2 NeuronCores.

Sharding: (batch b, image half) -> core 2*b + half. Each core:
  1. offset conv (PE matmuls over 2 c-tiles x 9 taps, N=400 position chunks),
     software-pipelined: rc0/rc1 -> block-0 index math -> rc2..rc7 -> rest,
     so block-0 gathers start ~35us in, under the conv tail
  2. transpose offsets to position-major; DVE index/bilinear-weight math;
     idx fold to the gather's 16-wrapped layout via 8 PE permute-matmuls
     (fp32 exact) + strided DVE copies -- all on-chip, no DMA bounce
  3. dma_gather of 2x2 fp16 pixel patches (all 256 ch) from an interleaved
     row-pair HBM image, positions-on-partitions; gather buffers live in a
     pool that coexists with the prologue pool (no SBUF-reuse barrier)
  4. bilinear combine on PE as diagonal-matmul accumulation
     (psum[c,p] += sum_j plane_j^T @ diag(beta_j)) -- transposes to
     channel-major for free; the 36 diag tiles per chunk are built in ONE
     stride-0-broadcast DVE op (dt_all = id16 * beta16)
  5. main conv = 18-chunk PE accumulation over (c-tile, tap); bias folded
     into the per-partition activation bias of the psum->SBUF copy; fp16
     output staging (host upcasts).
"""
import numpy as np

B, CIN, COUT, H, W = 4, 256, 256, 80, 80
NCORES = 8
HHALF = H // 2                      # 40 rows per core
NPOS = HHALF * W                    # 3200 positions per core
NCHUNK = NPOS // 128                # 25
PITCH = 84                          # x2 pixel-group pitch per row
NGROUPS = PITCH * PITCH             # 7056 (rows in x2; 83*84 + slack)
FBIAS = 16.0                        # float->int truncation bias (floor trick)
CLIP_LO = 14.0                      # = -2 + FBIAS
CLIP_HI = 96.9                      # = 80.9 + FBIAS
FLAT_OFF = -(14 * PITCH + 14)       # flat = y0b*84 + x0b + FLAT_OFF
BLOCKS = [(0, 512), (512, 512), (1024, 512), (1536, 512),
          (2048, 512), (2560, 512), (3072, 128)]
NT_PE = 9                           # taps 0..NT_PE-1 combine on PE (diag)

_cached = {}


def _build_program():
    from concourse import bass, bacc, tile, mybir
    from contextlib import ExitStack

    fp16, fp32 = mybir.dt.float16, mybir.dt.float32
    i16, i32 = mybir.dt.int16, mybir.dt.int32
    A = mybir.AluOpType
    ACT_COPY = mybir.ActivationFunctionType.Copy
    ACT_IDENT = mybir.ActivationFunctionType.Identity

    nc = bacc.Bacc("TRN2", target_bir_lowering=False, debug=False,
                   num_devices=NCORES, num_swdge_queues=4)

    x2_d = nc.dram_tensor("x2", [NGROUPS, 512], fp16, kind="ExternalInput")
    xcf_d = nc.dram_tensor("xcf", [128, 2, 44 * PITCH], fp16, kind="ExternalInput")
    offw_d = nc.dram_tensor("offw", [128, 2, 9, 18], fp16, kind="ExternalInput")
    offb_d = nc.dram_tensor("offb", [18, 1], fp32, kind="ExternalInput")
    convw_d = nc.dram_tensor("convw", [128, 2, 9, 256], fp16, kind="ExternalInput")
    convb_d = nc.dram_tensor("convb", [128, 2], fp32, kind="ExternalInput")
    cyb_d = nc.dram_tensor("cyb", [128, NCHUNK, 9], fp32, kind="ExternalInput")
    cxb_d = nc.dram_tensor("cxb", [128, NCHUNK, 9], fp32, kind="ExternalInput")
    id16_d = nc.dram_tensor("id16", [128, 128], fp16, kind="ExternalInput")
    id32_d = nc.dram_tensor("id32", [18, 18], fp32, kind="ExternalInput")
    perm_d = nc.dram_tensor("perm", [128, 8, 128], fp32, kind="ExternalInput")
    out_d = nc.dram_tensor("out", [2, 128, NPOS], fp16, kind="ExternalOutput")

    # overlapping gather-source view: [NGROUPS-1, 1024] with row stride 512
    x2_view = x2_d.ap().copy()
    v = x2_view.ap
    v[0] = [512, NGROUPS - 1]
    v[1] = [1, 1024]
    x2_view.ap = v

    def revec(ap, dims, extra_offset=0):
        """Rebuild an AP's dim list: dims = [(stride, num), ...]."""
        a = ap.copy()
        vv = a.ap
        while len(vv) > 1:
            vv.pop()
        vv[0] = list(dims[0])
        for d in dims[1:]:
            vv.append(list(d))
        a.ap = vv
        a.offset = a.offset + extra_offset
        return a

    with tile.TileContext(nc) as tc:
        with ExitStack() as ctx:
            persist = ctx.enter_context(tc.tile_pool(name="persist", bufs=1))
            idxw = persist.tile([128, 9, NPOS // 16], i16)
            beta16 = persist.tile([128, NCHUNK, 9, 4], fp16)
            id16 = persist.tile([128, 128], fp16)
            convw = persist.tile([128, 2, 9, 256], fp16)
            convb = persist.tile([128, 2], fp32)
            # gather buffers outlive the prologue pool so block-0 gathers
            # can run under the tail of the offset conv (no SBUF-reuse
            # barrier against prologue tiles)
            gbpool = ctx.enter_context(tc.tile_pool(name="gbp", bufs=1))

            # ---------------- prologue: offsets + indices ----------------
            with ExitStack() as pctx:
                ppool = pctx.enter_context(tc.tile_pool(name="pro", bufs=1))
                ppsum = pctx.enter_context(
                    tc.tile_pool(name="ppsum", bufs=2, space="PSUM"))

                xcf_a = ppool.tile([128, 2, 14 * PITCH], fp16)
                xcf_b = ppool.tile([128, 2, 33 * PITCH], fp16)
                offw = ppool.tile([128, 2, 9, 18], fp16)
                offb = ppool.tile([18, 1], fp32)
                id32 = ppool.tile([18, 18], fp32)
                cyb = ppool.tile([128, NCHUNK, 9], fp32)
                cxb = ppool.tile([128, NCHUNK, 9], fp32)
                perm = ppool.tile([128, 8, 128], fp32)
                off_sb = ppool.tile([18, NPOS], fp32)
                offT = ppool.tile([128, NCHUNK, 18], fp32)
                # conv-critical loads first; everything else after
                nc.sync.dma_start(out=offw[:], in_=offw_d[:])
                nc.sync.dma_start(out=offb[:], in_=offb_d[:])
                nc.sync.dma_start(out=xcf_a[:], in_=xcf_d[:, :, 0:14 * PITCH])
                nc.sync.dma_start(out=xcf_b[:],
                                  in_=xcf_d[:, :, 11 * PITCH:44 * PITCH])
                nc.sync.dma_start(out=id32[:], in_=id32_d[:])
                nc.sync.dma_start(out=cyb[:], in_=cyb_d[:])
                nc.sync.dma_start(out=cxb[:], in_=cxb_d[:])
                nc.sync.dma_start(out=perm[:], in_=perm_d[:])
                nc.sync.dma_start(out=id16[:], in_=id16_d[:])
                nc.sync.dma_start(out=convw[:], in_=convw_d[:])
                nc.sync.dma_start(out=convb[:], in_=convb_d[:])

                # offset conv: 8 chunks x 5 rows x 80 cols (N=400)
                # core's rows start at h0 (baked into cyb); xcf rows are
                # sample-global, so the row window ALSO must come from host:
                # we bake it by sending xcf pre-SLICED? No: xcf is full padded
                # image; row base differs per core. Use a per-core scalar via
                # cyb? Instead: host sends same xcf; the row offset is encoded
                # in a dram-input scalar-free way: we read rows via h0 from
                # cyb is not an AP offset. Trick: host sends xcf with the
                # core's 44-row window ALREADY positioned at a fixed place:
                # xcf layout = padded rows [h0-2 .. h0+42) relocated to rows
                # [0..44). See host prep. Conv output row r (0..39) reads
                # xcf rows (r + ky) in [0..42).
                NE = NCHUNK * 9   # 225
                pyb = ppool.tile([128, NE], fp32)
                pxb = ppool.tile([128, NE], fp32)
                t_i32 = ppool.tile([128, NE], i32)
                y0f = ppool.tile([128, NE], fp32)
                x0f = ppool.tile([128, NE], fp32)
                fy = ppool.tile([128, NE], fp32)
                fx = ppool.tile([128, NE], fp32)
                gy = ppool.tile([128, NE], fp32)
                gx = ppool.tile([128, NE], fp32)
                gtt = ppool.tile([128, NE], fp32)

                def conv_rc(rc):
                    ps = ppsum.tile([18, 400], fp32, tag="offps")
                    xt, roff, rows = ((xcf_a, 0, 14) if rc < 2 else
                                      (xcf_b, 11 * PITCH, 33))
                    mm = 0
                    for ct in range(2):
                        for t in range(9):
                            ky, kx = t // 3, t % 3
                            cb = (rc * 5 + ky + 1) * PITCH + kx + 1 - roff
                            mov = revec(xt[:, ct, 0],
                                        [(2 * rows * PITCH, 128), (PITCH, 5),
                                         (1, 80)],
                                        extra_offset=cb)
                            nc.tensor.matmul(
                                ps[:], lhsT=offw[:, ct, t, :], rhs=mov,
                                start=(mm == 0), stop=(mm == 17))
                            mm += 1
                    nc.scalar.activation(off_sb[:, rc * 400:(rc + 1) * 400],
                                         ps[:], ACT_IDENT, bias=offb[:])

                def transp(c0, c1):
                    # offsets [18, 128] -> [128, 18] per chunk
                    for ch in range(c0, c1):
                        ptx = ppsum.tile([128, 18], fp32, tag="offtps", bufs=1)
                        nc.tensor.matmul(ptx[:],
                                         lhsT=off_sb[:, ch * 128:(ch + 1) * 128],
                                         rhs=id32[:], start=True, stop=True,
                                         is_transpose=True)
                        nc.vector.tensor_copy(offT[:, ch, :], ptx[:])

                def math_fold(c0, c1):
                    """index/bilinear-weight math + idx fold, chunks [c0,c1)"""
                    nch = c1 - c0
                    ne = nch * 9
                    fw = lambda t: revec(t[:], [(NE, 128), (1, ne)], c0 * 9)
                    V = nc.vector
                    dyw = revec(offT[:], [(NCHUNK * 18, 128), (18, nch), (2, 9)],
                                c0 * 18)
                    dxw = revec(offT[:], [(NCHUNK * 18, 128), (18, nch), (2, 9)],
                                c0 * 18 + 1)
                    V.tensor_tensor(out=fw(pyb), in0=dyw, in1=cyb[:, c0:c1, :],
                                    op=A.add)
                    V.tensor_tensor(out=fw(pxb), in0=dxw, in1=cxb[:, c0:c1, :],
                                    op=A.add)
                    V.tensor_scalar(fw(pyb), fw(pyb), CLIP_LO, CLIP_HI,
                                    A.max, A.min)
                    V.tensor_scalar(fw(pxb), fw(pxb), CLIP_LO, CLIP_HI,
                                    A.max, A.min)
                    # robust floor: y0 = cvt(pyb); y0 -= (y0 > pyb)
                    V.tensor_copy(fw(t_i32), fw(pyb))
                    V.tensor_copy(fw(y0f), fw(t_i32))
                    V.tensor_tensor(out=fw(gtt), in0=fw(y0f), in1=fw(pyb),
                                    op=A.is_gt)
                    V.tensor_tensor(out=fw(y0f), in0=fw(y0f), in1=fw(gtt),
                                    op=A.subtract)
                    V.tensor_copy(fw(t_i32), fw(pxb))
                    V.tensor_copy(fw(x0f), fw(t_i32))
                    V.tensor_tensor(out=fw(gtt), in0=fw(x0f), in1=fw(pxb),
                                    op=A.is_gt)
                    V.tensor_tensor(out=fw(x0f), in0=fw(x0f), in1=fw(gtt),
                                    op=A.subtract)
                    V.tensor_tensor(out=fw(fy), in0=fw(pyb), in1=fw(y0f),
                                    op=A.subtract)
                    V.tensor_tensor(out=fw(fx), in0=fw(pxb), in1=fw(x0f),
                                    op=A.subtract)
                    # flat = (y0b*84 + x0b) - 1190
                    V.scalar_tensor_tensor(fw(pyb), fw(y0f), float(PITCH),
                                           fw(x0f), A.mult, A.add)
                    V.tensor_scalar_add(fw(pyb), fw(pyb), float(FLAT_OFF))
                    V.tensor_scalar(fw(gy), fw(fy), -1.0, 1.0, A.mult, A.add)
                    V.tensor_scalar(fw(gx), fw(fx), -1.0, 1.0, A.mult, A.add)
                    # beta: b0=gx*gy b1=gx*fy b2=fx*gy b3=fx*fy
                    v3w = lambda t: revec(t[:], [(NE, 128), (9, nch), (1, 9)],
                                          c0 * 9)
                    bjw = lambda j: revec(beta16[:], [(NE * 4, 128), (36, nch),
                                                      (4, 9)], c0 * 36 + j)
                    V.tensor_tensor(out=bjw(0), in0=v3w(gx), in1=v3w(gy), op=A.mult)
                    V.tensor_tensor(out=bjw(1), in0=v3w(gx), in1=v3w(fy), op=A.mult)
                    V.tensor_tensor(out=bjw(2), in0=v3w(fx), in1=v3w(gy), op=A.mult)
                    V.tensor_tensor(out=bjw(3), in0=v3w(fx), in1=v3w(fy), op=A.mult)
                    # idx fold on-chip: flat fp32 -> idxw i16,
                    # idxw[16q+r, k, ch*8+a] = flat[16a+r, ch, k] via PE
                    # permute-matmul (fp32 exact), psum->i32, strided i32->i16.
                    for a in range(8):
                        fpp = ppsum.tile([128, NE], fp32, tag="foldps", bufs=1)
                        nc.tensor.matmul(fpp[:, :ne], lhsT=perm[:, a, :],
                                         rhs=fw(pyb), start=True, stop=True)
                        f32 = ppool.tile([128, NE], i32, tag="ftmp", bufs=2)
                        nc.vector.tensor_copy(f32[:, :ne], fpp[:, :ne])
                        src = revec(f32[:], [(NE, 128), (9, nch), (1, 9)])
                        dst = revec(idxw[:], [(9 * 200, 128), (8, nch), (200, 9)],
                                    a + c0 * 8)
                        nc.vector.tensor_copy(dst, src)

                # pipelined prologue: block 0's indices first so its gathers
                # start while the rest of the offset conv still runs on PE
                conv_rc(0)
                conv_rc(1)
                transp(0, 4)
                math_fold(0, 4)
                for rc in range(2, 8):
                    conv_rc(rc)
                transp(4, NCHUNK)
                math_fold(4, NCHUNK)

            # ---------------- main loop ----------------
            with ExitStack() as mctx:
                mpool = mctx.enter_context(tc.tile_pool(name="main", bufs=1))
                mpsum = mctx.enter_context(
                    tc.tile_pool(name="mpsum", bufs=2, space="PSUM"))

                for bi, (base, npos) in enumerate(BLOCKS):
                    nsub = npos // 128
                    gbs = []
                    for k in range(9):
                        gb = gbpool.tile([128, 4, 1024], fp16, tag=f"gb{k}",
                                         bufs=(2 if k < 8 else 1))
                        nc.gpsimd.dma_gather(
                            gb[:, :nsub, :], x2_view,
                            idxw[:, k, base // 16:(base + npos) // 16],
                            npos, npos, 1024, elem_step=512,
                            queue_num=k % 4)
                        gbs.append(gb)

                    # valbuf s-major: [128, s, ci=k*2+ct, 128] so each
                    # (k, s) combine write is one contiguous [128, 256]
                    valbuf = mpool.tile([128, 4, 18, 128], fp16, tag="valbuf",
                                        bufs=2)
                    for s in range(nsub):
                        ch = base // 128 + s
                        # diag tiles, one stride-0 broadcast DVE op:
                        # dt_all[p, k*4+j, :] = id16[p, :] * beta16[p, ch, k, j]
                        dt_all = mpool.tile([128, 36, 128], fp16,
                                            tag="dtall", bufs=2)
                        nc.vector.tensor_tensor(
                            out=dt_all[:],
                            in0=revec(id16[:], [(128, 128), (0, 36), (1, 128)]),
                            in1=revec(beta16[:], [(NE * 4, 128), (1, 36),
                                                  (0, 128)], ch * 36),
                            op=A.mult)
                        # psum[c, p'] += sum_j gb_j^T @ diag(beta_j)
                        # (transpose + scale in one)
                        for k in range(9):
                            pv = mpsum.tile([128, 256], fp32, tag="pvb",
                                            bufs=4)
                            for ct in range(2):
                                for j in range(4):
                                    nc.tensor.matmul(
                                        pv[:, ct * 128:ct * 128 + 128],
                                        lhsT=gbs[k][:, s,
                                                    (2 * j + ct) * 128:
                                                    (2 * j + ct + 1) * 128],
                                        rhs=dt_all[:, k * 4 + j, :],
                                        start=(j == 0), stop=(j == 3))
                            nc.scalar.activation(
                                valbuf[:, s, k * 2:k * 2 + 2, :],
                                pv[:], ACT_COPY)

                    for ot in range(2):
                        po = mpsum.tile([128, 512], fp32, tag=f"po{ot}", bufs=2)
                        for ci in range(18):
                            k, ct = ci // 2, ci % 2
                            rhsv = revec(
                                valbuf[:], [(4 * 18 * 128, 128),
                                            (18 * 128, nsub), (1, 128)],
                                ci * 128)
                            nc.tensor.matmul(
                                po[:, :npos],
                                lhsT=convw[:, ct, k, ot * 128:(ot + 1) * 128],
                                rhs=rhsv,
                                start=(ci == 0), stop=(ci == 17))
                        osb = mpool.tile([128, 512], fp16, tag="osb", bufs=2)
                        nc.scalar.activation(osb[:, :npos], po[:, :npos],
                                             ACT_IDENT, bias=convb[:, ot:ot + 1])
                        nc.sync.dma_start(
                            out=out_d[ot, :, base:base + npos],
                            in_=osb[:, :npos])

    nc.compile()
    return nc


def _host_prep(x, offset_w, offset_b, conv_w, conv_b):
    """Build per-core input maps."""
    x = np.asarray(x, np.float32)
    offset_w = np.asarray(offset_w, np.float32)
    offset_b = np.asarray(offset_b, np.float32)
    conv_w = np.asarray(conv_w, np.float32)
    conv_b = np.asarray(conv_b, np.float32)

    # weights, shared
    # offset_w: [18, 256, 3, 3] -> [c128, ct, t, d]
    ow = offset_w.reshape(18, 2, 128, 3, 3)
    offw_h = np.ascontiguousarray(
        ow.reshape(18, 2, 128, 9).transpose(2, 1, 3, 0)).astype(np.float16)
    offb_h = offset_b.reshape(18, 1).astype(np.float32)
    cw = conv_w.reshape(256, 2, 128, 9)
    convw_h = np.ascontiguousarray(cw.transpose(2, 1, 3, 0)).astype(np.float16)  # [c,ct,t,o]
    convb_h = np.ascontiguousarray(conv_b.reshape(2, 128).T).astype(np.float32)
    id16_h = np.eye(128, dtype=np.float16)
    id32_h = np.eye(18, dtype=np.float32)
    # fold permutation: perm[a][16a+r, 16q+r] = 1  (lhsT for out_a = P_a @ flat)
    perm_h = np.zeros((128, 8, 128), np.float32)
    for a in range(8):
        for r in range(16):
            for q in range(8):
                perm_h[16 * a + r, a, 16 * q + r] = 1.0

    # per-core base constants
    k = np.arange(9)
    ry = (k // 3 - 1).astype(np.float32)
    rx = (k % 3 - 1).astype(np.float32)
    in_maps = []
    per_sample = {}
    for b in range(B):
        xc = np.ascontiguousarray(x[b].transpose(1, 2, 0))       # [H, W, C]
        xp = np.pad(xc, ((2, 2), (2, 2), (0, 0))).astype(np.float16)  # [84, 84, 256]
        x2 = np.zeros((PITCH, PITCH, 2, 256), np.float16)
        x2[:83, :, 0] = xp[:83]
        x2[:83, :, 1] = xp[1:84]
        x2_h = x2.reshape(NGROUPS, 512)
        per_sample[b] = (x2_h, xp)

    for core in range(NCORES):
        b, half = core // 2, core % 2
        h0 = half * HHALF
        x2_h, xp = per_sample[b]
        # xcf: channel-first, rows [h0-2 .. h0+42) of the padded image
        # relocated to local rows [0..44): xcf[c, r, x'] = xpad_cf[c, h0+r, x']
        # padded row index of original row y is y+2; window r=0 -> orig h0-2.
        xcf_rows = xp[h0:h0 + 44]                                # [44, 84, 256]
        xcf_h = np.ascontiguousarray(
            xcf_rows.transpose(2, 0, 1).reshape(2, 128, 44 * PITCH)
            .transpose(1, 0, 2))

        i = np.arange(NPOS)
        hloc = i // W
        wloc = i % W
        cyb_h = ((h0 + hloc)[:, None] + ry[None, :] + FBIAS).astype(np.float32)
        cxb_h = (wloc[:, None] + rx[None, :] + FBIAS).astype(np.float32)
        cyb_h = np.ascontiguousarray(
            cyb_h.reshape(NCHUNK, 128, 9).transpose(1, 0, 2))
        cxb_h = np.ascontiguousarray(
            cxb_h.reshape(NCHUNK, 128, 9).transpose(1, 0, 2))

        in_maps.append({
            "x2": x2_h, "xcf": xcf_h, "offw": offw_h, "offb": offb_h,
            "convw": convw_h, "convb": convb_h, "cyb": cyb_h, "cxb": cxb_h,
            "id16": id16_h, "id32": id32_h, "perm": perm_h,
        })
    return in_maps


def kernel(x, offset_w, offset_b, conv_w, conv_b, _trace=False):
    from concourse.bass_utils import run_bass_kernel_spmd

    if "nc" not in _cached:
        _cached["nc"] = _build_program()
    nc = _cached["nc"]
    in_maps = _host_prep(x, offset_w, offset_b, conv_w, conv_b)
    res = run_bass_kernel_spmd(nc, in_maps, list(range(NCORES)), trace=_trace)
    _cached["last_result"] = res
    out = np.zeros((B, COUT, H, W), np.float32)
    for core in range(NCORES):
        b, half = core // 2, core % 2
        o = res.results[core]["out"]          # [2, 128, NPOS]
        out[b, :, half * HHALF:(half + 1) * HHALF, :] = \
            o.reshape(COUT, HHALF, W)
    return out

